# revision 9
# baseline (speedup 1.0000x reference)
"""Trainium2 Bass kernel for nn_BinaryLinear (binary-weight linear + BatchNorm + sign).

Computation (reference):
    bw    = sign(W)                     # [O, I], entries in {-1, 0, +1}
    alpha = mean(|W|, axis=1)           # [O]
    y     = x @ (bw * alpha).T          # [B, O]
    out   = sign((y - mu_b) / sqrt(var_b + eps) * gamma + beta)   # batch stats

Strategy (8 NeuronCores, column-sharded):
  * Each core owns O/8 = 512 output columns; BN batch stats are then fully
    local to a core (full batch for its columns) -> no collectives.
  * alpha is factored out of the matmul: s = x @ bw.T runs on the PE in a
    SINGLE fp16 pass (weights +-1 exact in fp16; x quantized to fp16 =
    ~12.2 mantissa bits, HW-measured d_rms 2.08e-4). The induced sign-flip
    count stays well under the rel-err gate, and PE work HALVES vs a bf16
    hi/lo split: 216 ns per [K=128, N=512] matmul warm.
  * fp16 also halves every buffer: weights are 32K/part resident (no
    casts), x tiles are 1K/part -> a 32-deep x ring (~28us of runway)
    rides out all DMA jitter at only ~150 GB/s demand.
  * Layout is transposed on host: s.T[o, b] so that o sits on SBUF
    partitions. BN stats are per-partition reductions along the free dim
    (BN_STATS/BN_AGGR); the final affine+sign drains on ScalarE (Sign,
    +-1) and DVE (is_ge, {0,1}; host decodes 2b-1) slabs writing int8.
"""

import os
from contextlib import ExitStack

import ml_dtypes
import numpy as np

import concourse.bacc as bacc
import concourse.bass as bass
import concourse.mybir as mybir
import concourse.tile as tile
from concourse.bass_utils import run_bass_kernel_spmd

BN_EPS = 1e-5

N_CORES = 8
B_FULL, IN_F, OUT_F = 8192, 4096, 4096

LAST_RESULTS = None  # BassKernelResults of the most recent device run


def slab_plan(B, NOT, CH, simple_tail):
    """Emission-ordered sign-slab plan: (ot, start, width, src, eng).

    src 'y' reads the SBUF y buffer; 'ps' reads the last chunk's PSUM bank
    directly (its evacuation copy is skipped). eng 'dve' emits {0,1} via
    is_ge (host decodes 2b-1); 'act' emits +-1 via ACTIVATE(Sign).
    Split is balanced for measured rates: DVE 0.585 ns/elem (1 op),
    ACT 1.02 ns/elem, DVE carrying the extra stats/coef prelude.
    """
    plan = []
    if simple_tail and B == 8192 and NOT == 4 and CH == 512:
        ACT_FULL = {1, 2, 4, 5, 8, 11}
        for h in range(3):
            for ot in range(NOT):
                k = h * NOT + ot
                eng = "act" if k in ACT_FULL else "dve"
                plan.append((ot, h * 2048, 2048, "y", eng))
        for ot in range(NOT):
            plan.append((ot, 6144, 1536, "y", "act" if ot == 0 else "dve"))
        for ot in range(NOT):
            plan.append((ot, 7680, 512, "ps", "dve"))
        return plan, True
    # generic fallback: equal 2048-wide y slabs, ACT every 3rd (simple)
    # or every 2nd (full BN affine)
    SGW = min(B, 2048)
    for h in range(B // SGW):
        for ot in range(NOT):
            k = h * NOT + ot
            if simple_tail:
                eng = "act" if k % 3 == 2 else "dve"
            else:
                eng = "act" if k % 2 == 1 else "dve"
            plan.append((ot, h * SGW, SGW, "y", eng))
    return plan, False


def build_nc(B, I, OSH, CH=512, xbufs=32, simple_tail=False):
    """Build + compile the per-core Bass program.

    B: batch (free dim of s.T), I: contraction, OSH: output columns per core,
    CH: batch chunk (<=512, PSUM bank / bn_stats limit). simple_tail may only
    be set when gamma > 0 and beta == 0 (sign(BN(y)) == sign(s - mean_s)).
    """
    NOT = OSH // 128          # o-tiles (PSUM partition groups)
    NT = I // 128             # i-tiles (contraction)
    NCH = B // CH             # batch chunks
    f32 = mybir.dt.float32
    fp16 = mybir.dt.float16
    bf16 = mybir.dt.bfloat16
    i8 = mybir.dt.int8

    nc = bacc.Bacc("TRN2", target_bir_lowering=False, debug=False)
    xtp_d = nc.dram_tensor(
        "xtp", [NT, NCH, 128, CH], fp16, kind="ExternalInput"
    )
    bwt_d = nc.dram_tensor("bwt", [NT, 128, OSH], fp16, kind="ExternalInput")
    coef_d = nc.dram_tensor("coef", [128, 4 * NOT], f32, kind="ExternalInput")
    out_d = nc.dram_tensor("out", [OSH, B], i8, kind="ExternalOutput")
    SGW = min(B, 2048)        # sign-pass slab width
    NSG = B // SGW

    with tile.TileContext(nc) as tc, ExitStack() as ctx:
        w_pool = ctx.enter_context(tc.tile_pool(name="w", bufs=NT))
        x_pool = ctx.enter_context(tc.tile_pool(name="x", bufs=xbufs))
        sg_pool = ctx.enter_context(tc.tile_pool(name="sg", bufs=6))
        y_pool = ctx.enter_context(tc.tile_pool(name="y", bufs=1))
        ps_pool = ctx.enter_context(
            tc.tile_pool(name="ps", bufs=8, space=bass.MemorySpace.PSUM)
        )
        st_pool = ctx.enter_context(tc.tile_pool(name="st", bufs=1))
        sm_pool = ctx.enter_context(tc.tile_pool(name="sm", bufs=NOT))

        # PE warm-up: the HAM clock gate holds the PE at 1.2 GHz until it has
        # been busy ~3.4us. Burn dummy matmuls during the initial DMA wait so
        # the real matmul stream starts at 2.4 GHz.
        wl = sm_pool.tile([128, 8], bf16)
        wr = sm_pool.tile([128, 8], bf16)
        nc.vector.memset(wl[:], 0.0)
        nc.vector.memset(wr[:], 0.0)
        wp = ps_pool.tile([128, CH], f32, name="wups", tag="ps")
        for _ in range(64):
            nc.tensor.matmul(wp[0:8, 0:8], wl[:], wr[:], start=True, stop=True)

        # fp16 weight tiles (128KB each); DMAs are issued inside the chunk-0
        # loop, interleaved with the x stream on the opposite HWDGE ring, so
        # weight t arrives just ahead of its first use instead of queueing
        # megabytes ahead of x in the ring FIFO.
        w_tiles = [None] * NT
        plan, psum_direct = slab_plan(B, NOT, CH, simple_tail)

        ct = st_pool.tile([128, 4 * NOT], f32)
        nc.gpsimd.dma_start(ct[:], coef_d.ap())

        yt = [y_pool.tile([128, B], f32, name=f"yt{i}") for i in range(NOT)]
        stats = [st_pool.tile([128, 6 * NCH], f32, name=f"stats{i}") for i in range(NOT)]

        # Per-o-tile BN coefficients: with s-stats (mean_s, var_s) and host
        # precomputed p1=alpha^2, p2=alpha*gamma, p4=beta:
        #   inv = 1/sqrt(p1*var_s + eps);  A = p2*inv;  B = p4 - mean_s*A
        A_t, B_t, mv_t = [None] * NOT, [None] * NOT, [None] * NOT
        eps_t = sm_pool.tile([128, 1], f32)
        nc.vector.memset(eps_t[:], BN_EPS)

        def coef_chain(ot):
            mv = sm_pool.tile([128, 2], f32, name=f"mv{ot}", tag="mv")
            nc.vector.bn_aggr(mv[:], stats[ot][:])
            p1 = ct[:, ot : ot + 1]
            p2 = ct[:, NOT + ot : NOT + ot + 1]
            p4 = ct[:, 3 * NOT + ot : 3 * NOT + ot + 1]
            v = sm_pool.tile([128, 1], f32, name=f"v{ot}", tag="v")
            nc.vector.tensor_mul(v[:], mv[:, 1:2], p1)
            sd = sm_pool.tile([128, 1], f32, name=f"sd{ot}", tag="sd")
            nc.scalar.activation(
                sd[:], v[:], mybir.ActivationFunctionType.Sqrt, bias=eps_t[:]
            )
            inv = sm_pool.tile([128, 1], f32, name=f"inv{ot}", tag="inv")
            nc.vector.reciprocal(inv[:], sd[:])
            Ac = sm_pool.tile([128, 1], f32, name=f"Ac{ot}", tag="Ac")
            nc.vector.tensor_mul(Ac[:], p2, inv[:])
            mB = sm_pool.tile([128, 1], f32, name=f"mB{ot}", tag="mB")
            nc.vector.tensor_mul(mB[:], mv[:, 0:1], Ac[:])
            Bc = sm_pool.tile([128, 1], f32, name=f"Bc{ot}", tag="Bc")
            nc.vector.tensor_sub(Bc[:], p4, mB[:])
            A_t[ot], B_t[ot], mv_t[ot] = Ac, Bc, mv

        for c in range(NCH):
            if c == 1:
                # Preload the tail ACT LUTs (Sqrt, Sign) once the stream is
                # rolling: no ACT_TABLE_LOAD on the critical tail, and no
                # delay to the startup DMA issues on the ACT ring.
                wt = sm_pool.tile([128, 1], f32)
                nc.vector.memset(wt[:], 1.0)
                wt2 = sm_pool.tile([128, 1], f32)
                nc.scalar.activation(
                    wt2[:], wt[:], mybir.ActivationFunctionType.Sqrt
                )
                nc.scalar.activation(
                    wt2[:], wt[:], mybir.ActivationFunctionType.Sign
                )
            ps = [ps_pool.tile([128, CH], f32, name=f"ps{c}_{i}", tag="ps") for i in range(NOT)]
            if c == NCH - 1:
                last_ps = ps
            for t in range(NT):
                xt = x_pool.tile([128, CH], fp16, tag="x")
                # single-tile DMAs on alternating HWDGE rings (SP / ACT)
                dma_eng = nc.sync if t % 2 == 0 else nc.scalar
                dma_eng.dma_start(xt[:], xtp_d.ap()[t, c])
                if c == 0:
                    w = w_pool.tile([128, OSH], fp16, name=f"w{t}", tag="w")
                    w_eng = nc.scalar if t % 2 == 0 else nc.sync
                    w_eng.dma_start(w[:], bwt_d.ap()[t])
                    w_tiles[t] = w
                for ot in range(NOT):
                    lhsT = w_tiles[t][:, ot * 128 : (ot + 1) * 128]
                    nc.tensor.matmul(
                        ps[ot][:], lhsT, xt[:, 0:CH],
                        start=(t == 0), stop=(t == NT - 1),
                    )
            for ot in range(NOT):
                ysl = yt[ot][:, c * CH : (c + 1) * CH]
                # stats read PSUM directly so the tail-critical chain does not
                # wait on the ACT evacuation copy
                nc.vector.bn_stats(stats[ot][:, c * 6 : (c + 1) * 6], ps[ot][:])
                if c == NCH - 1:
                    # emit the coefficient chain right after this o-tile's
                    # final stats so the first sign slab starts ASAP
                    coef_chain(ot)
                    if not psum_direct:
                        nc.scalar.copy(ysl, ps[ot][:])
                else:
                    nc.vector.tensor_copy(ysl, ps[ot][:])

        # Final affine+sign, split between ScalarE (ACTIVATE(Sign), +-1)
        # and DVE (is_ge, {0,1}) per the shared slab plan; the last batch
        # chunk is signed straight out of PSUM (no evacuation copy), and the
        # small PSUM slabs come last so the final out-DMA is tiny.
        for si, (ot, st, wd, srk, eng) in enumerate(plan):
            ysl = (last_ps[ot][:, st - (NCH - 1) * CH : st - (NCH - 1) * CH + wd]
                   if srk == "ps" else yt[ot][:, st : st + wd])
            sg = sg_pool.tile([128, SGW], i8, name=f"sg{si}", tag="sg")
            if eng == "dve":
                # DVE slabs emit b in {0,1}; the host decodes 2b-1.
                if simple_tail:
                    # sign(BN(y)) == 2*(s >= mean_s) - 1
                    nc.vector.tensor_scalar(
                        sg[:, 0:wd], ysl, mv_t[ot][:, 0:1], None,
                        mybir.AluOpType.is_ge,
                    )
                else:
                    # y' = y*A + B; b = (y' >= 0)
                    nc.vector.tensor_scalar(
                        ysl, ysl, A_t[ot][:], B_t[ot][:],
                        mybir.AluOpType.mult, mybir.AluOpType.add,
                    )
                    nc.vector.tensor_scalar(
                        sg[:, 0:wd], ysl, 0.0, None, mybir.AluOpType.is_ge
                    )
            else:
                nc.scalar.activation(
                    sg[:, 0:wd], ysl,
                    mybir.ActivationFunctionType.Sign,
                    bias=B_t[ot][:],
                    scale=A_t[ot][:],
                )
            nc.sync.dma_start(
                out_d.ap()[ot * 128 : (ot + 1) * 128, st : st + wd],
                sg[:, 0:wd],
            )

    nc.compile()
    return nc


def prep_inputs(x, w, gamma, beta, n_cores=N_CORES, CH=512):
    """Host-side prep: transpose/chunk x (fp16), bw/coef shards per core."""
    B, I = x.shape
    O = w.shape[0]
    OSH = O // n_cores
    NT = I // 128
    NCH = B // CH

    # [I, B] -> [NT, NCH, 128, CH]
    xt = np.ascontiguousarray(x.T)                # [I, B]
    xtp = np.ascontiguousarray(
        xt.reshape(NT, 128, NCH, CH).transpose(0, 2, 1, 3).astype(np.float16)
    )

    bw = np.sign(w).astype(np.float32)
    alpha = np.abs(w).mean(axis=1)                 # [O] f32
    p1 = alpha * alpha
    p2 = alpha * gamma
    p3 = alpha * alpha * gamma
    p4 = beta.astype(np.float32)

    in_maps = []
    for k in range(n_cores):
        sl = slice(k * OSH, (k + 1) * OSH)
        bwt = np.ascontiguousarray(bw[sl].T).reshape(NT, 128, OSH)
        NOT = OSH // 128

        def per_tile(vec):
            return np.ascontiguousarray(vec[sl].reshape(NOT, 128).T)  # [128, NOT]

        coef = np.concatenate(
            [per_tile(p1), per_tile(p2), per_tile(p3), per_tile(p4)], axis=1
        ).astype(np.float32)
        in_maps.append({
            "xtp": xtp,
            "bwt": bwt.astype(np.float16),
            "coef": coef,
        })
    return in_maps


_NC_CACHE = {}


def kernel(x, real_weight, gamma, beta):
    global LAST_RESULTS
    x = np.asarray(x, dtype=np.float32)
    w = np.asarray(real_weight, dtype=np.float32)
    gamma = np.asarray(gamma, dtype=np.float32)
    beta = np.asarray(beta, dtype=np.float32)
    B, I = x.shape
    O = w.shape[0]
    OSH = O // N_CORES
    CH = 512

    simple_tail = bool((gamma > 0).all() and (beta == 0).all())
    key = (B, I, OSH, CH, simple_tail)
    if key not in _NC_CACHE:
        _NC_CACHE[key] = build_nc(B, I, OSH, CH, simple_tail=simple_tail)
    nc = _NC_CACHE[key]

    in_maps = prep_inputs(x, w, gamma, beta, N_CORES, CH)
    trace = bool(int(os.environ.get("KERNEL_TRACE", "0")))
    res = run_bass_kernel_spmd(
        nc, in_maps, core_ids=list(range(N_CORES)), trace=trace
    )
    LAST_RESULTS = res

    NOT = OSH // 128
    plan, _ = slab_plan(B, NOT, CH, simple_tail)
    out = np.empty((B, O), dtype=np.float32)
    for k in range(N_CORES):
        o = res.results[k]["out"].astype(np.float32)   # [OSH, B] int8
        for ot, st, wd, srk, eng in plan:
            if eng != "dve":
                continue
            slb = o[ot * 128 : (ot + 1) * 128, st : st + wd]
            np.multiply(slb, 2.0, out=slb)
            np.subtract(slb, 1.0, out=slb)
        out[:, k * OSH : (k + 1) * OSH] = o.T
    return out


# revision 11
# speedup vs baseline: 1.0053x; 1.0053x over previous
"""Trainium2 Bass kernel for nn_BinaryLinear (binary-weight linear + BatchNorm + sign).

Computation (reference):
    bw    = sign(W)                     # [O, I], entries in {-1, 0, +1}
    alpha = mean(|W|, axis=1)           # [O]
    y     = x @ (bw * alpha).T          # [B, O]
    out   = sign((y - mu_b) / sqrt(var_b + eps) * gamma + beta)   # batch stats

Strategy (8 NeuronCores, column-sharded):
  * Each core owns O/8 = 512 output columns; BN batch stats are then fully
    local to a core (full batch for its columns) -> no collectives.
  * alpha is factored out of the matmul: s = x @ bw.T runs on the PE in a
    SINGLE fp16 pass (weights +-1 exact in fp16; x quantized to fp16 =
    ~12.2 mantissa bits, HW-measured d_rms 2.08e-4). The induced sign-flip
    count stays well under the rel-err gate, and PE work HALVES vs a bf16
    hi/lo split: 216 ns per [K=128, N=512] matmul warm.
  * fp16 also halves every buffer: weights are 32K/part resident (no
    casts), x tiles are 1K/part -> a 32-deep x ring (~28us of runway)
    rides out all DMA jitter at only ~150 GB/s demand.
  * Layout is transposed on host: s.T[o, b] so that o sits on SBUF
    partitions. BN stats are per-partition reductions along the free dim
    (BN_STATS/BN_AGGR); the final affine+sign drains on ScalarE (Sign,
    +-1) and DVE (is_ge, {0,1}; host decodes 2b-1) slabs writing int8.
"""

import os
from contextlib import ExitStack

import ml_dtypes
import numpy as np

import concourse.bacc as bacc
import concourse.bass as bass
import concourse.mybir as mybir
import concourse.tile as tile
from concourse.bass_utils import run_bass_kernel_spmd

BN_EPS = 1e-5

N_CORES = 8
B_FULL, IN_F, OUT_F = 8192, 4096, 4096

LAST_RESULTS = None  # BassKernelResults of the most recent device run


def slab_plan(B, NOT, CH, simple_tail):
    """Emission-ordered sign-slab plan: (ot, start, width, src, eng).

    src 'y' reads the SBUF y buffer; 'ps' reads the last chunk's PSUM bank
    directly (its evacuation copy is skipped). eng 'dve' emits {0,1} via
    is_ge (host decodes 2b-1); 'act' emits +-1 via ACTIVATE(Sign).
    Split is balanced for measured rates: DVE 0.585 ns/elem (1 op),
    ACT 1.02 ns/elem, DVE carrying the extra stats/coef prelude.
    """
    plan = []
    if simple_tail and B == 8192 and NOT == 4 and CH == 512:
        # Readiness-ordered: each engine's first slabs are low-ot (whose BN
        # coefficient chains finish first). ACT ~6.75 full-slab units from
        # T0+1.3us, DVE ~9.25 units from T0+4.8us (after stats+chains) ->
        # both engines drain together.
        H = 2048
        plan = [
            (0, 0 * H, H, "y", "act"), (1, 0 * H, H, "y", "act"),
            (0, 2 * H, H, "y", "dve"), (2, 0 * H, H, "y", "act"),
            (1, 2 * H, H, "y", "dve"), (3, 0 * H, H, "y", "act"),
            (2, 1 * H, H, "y", "dve"), (3, 1 * H, H, "y", "dve"),
            (0, 1 * H, H, "y", "act"), (2, 2 * H, H, "y", "dve"),
            (1, 1 * H, H, "y", "act"), (3, 2 * H, H, "y", "dve"),
            (0, 6144, 1536, "y", "dve"), (2, 6144, 1536, "y", "act"),
            (1, 6144, 1536, "y", "dve"), (3, 6144, 1536, "y", "dve"),
            (0, 7680, 512, "ps", "dve"), (1, 7680, 512, "ps", "dve"),
            (2, 7680, 512, "ps", "dve"), (3, 7680, 512, "ps", "dve"),
        ]
        return plan, True
    # generic fallback: equal 2048-wide y slabs, ACT every 3rd (simple)
    # or every 2nd (full BN affine)
    SGW = min(B, 2048)
    for h in range(B // SGW):
        for ot in range(NOT):
            k = h * NOT + ot
            if simple_tail:
                eng = "act" if k % 3 == 2 else "dve"
            else:
                eng = "act" if k % 2 == 1 else "dve"
            plan.append((ot, h * SGW, SGW, "y", eng))
    return plan, False


def build_nc(B, I, OSH, CH=512, xbufs=32, simple_tail=False):
    """Build + compile the per-core Bass program.

    B: batch (free dim of s.T), I: contraction, OSH: output columns per core,
    CH: batch chunk (<=512, PSUM bank / bn_stats limit). simple_tail may only
    be set when gamma > 0 and beta == 0 (sign(BN(y)) == sign(s - mean_s)).
    """
    NOT = OSH // 128          # o-tiles (PSUM partition groups)
    NT = I // 128             # i-tiles (contraction)
    NCH = B // CH             # batch chunks
    f32 = mybir.dt.float32
    fp16 = mybir.dt.float16
    bf16 = mybir.dt.bfloat16
    i8 = mybir.dt.int8

    nc = bacc.Bacc("TRN2", target_bir_lowering=False, debug=False)
    xtp_d = nc.dram_tensor(
        "xtp", [NT, NCH, 128, CH], fp16, kind="ExternalInput"
    )
    bwt_d = nc.dram_tensor("bwt", [NT, 128, OSH], fp16, kind="ExternalInput")
    coef_d = nc.dram_tensor("coef", [128, 4 * NOT], f32, kind="ExternalInput")
    out_d = nc.dram_tensor("out", [OSH, B], i8, kind="ExternalOutput")
    SGW = min(B, 2048)        # sign-pass slab width
    NSG = B // SGW

    with tile.TileContext(nc) as tc, ExitStack() as ctx:
        w_pool = ctx.enter_context(tc.tile_pool(name="w", bufs=NT))
        x_pool = ctx.enter_context(tc.tile_pool(name="x", bufs=xbufs))
        sg_pool = ctx.enter_context(tc.tile_pool(name="sg", bufs=6))
        y_pool = ctx.enter_context(tc.tile_pool(name="y", bufs=1))
        ps_pool = ctx.enter_context(
            tc.tile_pool(name="ps", bufs=8, space=bass.MemorySpace.PSUM)
        )
        st_pool = ctx.enter_context(tc.tile_pool(name="st", bufs=1))
        sm_pool = ctx.enter_context(tc.tile_pool(name="sm", bufs=NOT))

        # PE warm-up: the HAM clock gate holds the PE at 1.2 GHz until it has
        # been busy ~3.4us. Burn dummy matmuls during the initial DMA wait so
        # the real matmul stream starts at 2.4 GHz.
        wl = sm_pool.tile([128, 8], bf16)
        wr = sm_pool.tile([128, 8], bf16)
        nc.vector.memset(wl[:], 0.0)
        nc.vector.memset(wr[:], 0.0)
        wp = ps_pool.tile([128, CH], f32, name="wups", tag="ps")
        for _ in range(64):
            nc.tensor.matmul(wp[0:8, 0:8], wl[:], wr[:], start=True, stop=True)

        # fp16 weight tiles (128KB each); DMAs are issued inside the chunk-0
        # loop, interleaved with the x stream on the opposite HWDGE ring, so
        # weight t arrives just ahead of its first use instead of queueing
        # megabytes ahead of x in the ring FIFO.
        w_tiles = [None] * NT
        plan, psum_direct = slab_plan(B, NOT, CH, simple_tail)

        ct = st_pool.tile([128, 4 * NOT], f32)
        nc.gpsimd.dma_start(ct[:], coef_d.ap())

        yt = [y_pool.tile([128, B], f32, name=f"yt{i}") for i in range(NOT)]
        stats = [st_pool.tile([128, 6 * NCH], f32, name=f"stats{i}") for i in range(NOT)]

        # Per-o-tile BN coefficients: with s-stats (mean_s, var_s) and host
        # precomputed p1=alpha^2, p2=alpha*gamma, p4=beta:
        #   inv = 1/sqrt(p1*var_s + eps);  A = p2*inv;  B = p4 - mean_s*A
        A_t, B_t, mv_t = [None] * NOT, [None] * NOT, [None] * NOT
        eps_t = sm_pool.tile([128, 1], f32)
        nc.vector.memset(eps_t[:], BN_EPS)

        def coef_chain(ot):
            mv = sm_pool.tile([128, 2], f32, name=f"mv{ot}", tag="mv")
            nc.vector.bn_aggr(mv[:], stats[ot][:])
            p1 = ct[:, ot : ot + 1]
            p2 = ct[:, NOT + ot : NOT + ot + 1]
            p4 = ct[:, 3 * NOT + ot : 3 * NOT + ot + 1]
            v = sm_pool.tile([128, 1], f32, name=f"v{ot}", tag="v")
            nc.vector.tensor_mul(v[:], mv[:, 1:2], p1)
            sd = sm_pool.tile([128, 1], f32, name=f"sd{ot}", tag="sd")
            nc.scalar.activation(
                sd[:], v[:], mybir.ActivationFunctionType.Sqrt, bias=eps_t[:]
            )
            inv = sm_pool.tile([128, 1], f32, name=f"inv{ot}", tag="inv")
            nc.vector.reciprocal(inv[:], sd[:])
            Ac = sm_pool.tile([128, 1], f32, name=f"Ac{ot}", tag="Ac")
            nc.vector.tensor_mul(Ac[:], p2, inv[:])
            mB = sm_pool.tile([128, 1], f32, name=f"mB{ot}", tag="mB")
            nc.vector.tensor_mul(mB[:], mv[:, 0:1], Ac[:])
            Bc = sm_pool.tile([128, 1], f32, name=f"Bc{ot}", tag="Bc")
            nc.vector.tensor_sub(Bc[:], p4, mB[:])
            A_t[ot], B_t[ot], mv_t[ot] = Ac, Bc, mv

        for c in range(NCH):
            if c == 1:
                # Preload the tail ACT LUTs (Sqrt, Sign) once the stream is
                # rolling: no ACT_TABLE_LOAD on the critical tail, and no
                # delay to the startup DMA issues on the ACT ring.
                wt = sm_pool.tile([128, 1], f32)
                nc.vector.memset(wt[:], 1.0)
                wt2 = sm_pool.tile([128, 1], f32)
                nc.scalar.activation(
                    wt2[:], wt[:], mybir.ActivationFunctionType.Sqrt
                )
                nc.scalar.activation(
                    wt2[:], wt[:], mybir.ActivationFunctionType.Sign
                )
            ps = [ps_pool.tile([128, CH], f32, name=f"ps{c}_{i}", tag="ps") for i in range(NOT)]
            if c == NCH - 1:
                last_ps = ps
            for t in range(NT):
                xt = x_pool.tile([128, CH], fp16, tag="x")
                # single-tile DMAs on alternating HWDGE rings (SP / ACT)
                dma_eng = nc.sync if t % 2 == 0 else nc.scalar
                dma_eng.dma_start(xt[:], xtp_d.ap()[t, c])
                if c == 0:
                    w = w_pool.tile([128, OSH], fp16, name=f"w{t}", tag="w")
                    w_eng = nc.scalar if t % 2 == 0 else nc.sync
                    w_eng.dma_start(w[:], bwt_d.ap()[t])
                    w_tiles[t] = w
                for ot in range(NOT):
                    lhsT = w_tiles[t][:, ot * 128 : (ot + 1) * 128]
                    nc.tensor.matmul(
                        ps[ot][:], lhsT, xt[:, 0:CH],
                        start=(t == 0), stop=(t == NT - 1),
                    )
            for ot in range(NOT):
                ysl = yt[ot][:, c * CH : (c + 1) * CH]
                # stats read PSUM directly so the tail-critical chain does not
                # wait on the ACT evacuation copy
                nc.vector.bn_stats(stats[ot][:, c * 6 : (c + 1) * 6], ps[ot][:])
                if c == NCH - 1:
                    # emit the coefficient chain right after this o-tile's
                    # final stats so the first sign slab starts ASAP
                    coef_chain(ot)
                    if not psum_direct:
                        nc.scalar.copy(ysl, ps[ot][:])
                else:
                    nc.vector.tensor_copy(ysl, ps[ot][:])

        # Final affine+sign, split between ScalarE (ACTIVATE(Sign), +-1)
        # and DVE (is_ge, {0,1}) per the shared slab plan; the last batch
        # chunk is signed straight out of PSUM (no evacuation copy), and the
        # small PSUM slabs come last so the final out-DMA is tiny.
        for si, (ot, st, wd, srk, eng) in enumerate(plan):
            ysl = (last_ps[ot][:, st - (NCH - 1) * CH : st - (NCH - 1) * CH + wd]
                   if srk == "ps" else yt[ot][:, st : st + wd])
            sg = sg_pool.tile([128, SGW], i8, name=f"sg{si}", tag="sg")
            if eng == "dve":
                # DVE slabs emit b in {0,1}; the host decodes 2b-1.
                if simple_tail:
                    # sign(BN(y)) == 2*(s >= mean_s) - 1
                    nc.vector.tensor_scalar(
                        sg[:, 0:wd], ysl, mv_t[ot][:, 0:1], None,
                        mybir.AluOpType.is_ge,
                    )
                else:
                    # y' = y*A + B; b = (y' >= 0)
                    nc.vector.tensor_scalar(
                        ysl, ysl, A_t[ot][:], B_t[ot][:],
                        mybir.AluOpType.mult, mybir.AluOpType.add,
                    )
                    nc.vector.tensor_scalar(
                        sg[:, 0:wd], ysl, 0.0, None, mybir.AluOpType.is_ge
                    )
            else:
                nc.scalar.activation(
                    sg[:, 0:wd], ysl,
                    mybir.ActivationFunctionType.Sign,
                    bias=B_t[ot][:],
                    scale=A_t[ot][:],
                )
            out_eng = nc.sync if eng == "act" else nc.gpsimd
            out_eng.dma_start(
                out_d.ap()[ot * 128 : (ot + 1) * 128, st : st + wd],
                sg[:, 0:wd],
            )

    nc.compile()
    return nc


def prep_inputs(x, w, gamma, beta, n_cores=N_CORES, CH=512):
    """Host-side prep: transpose/chunk x (fp16), bw/coef shards per core."""
    B, I = x.shape
    O = w.shape[0]
    OSH = O // n_cores
    NT = I // 128
    NCH = B // CH

    # [I, B] -> [NT, NCH, 128, CH]
    xt = np.ascontiguousarray(x.T)                # [I, B]
    xtp = np.ascontiguousarray(
        xt.reshape(NT, 128, NCH, CH).transpose(0, 2, 1, 3).astype(np.float16)
    )

    bw = np.sign(w).astype(np.float32)
    alpha = np.abs(w).mean(axis=1)                 # [O] f32
    p1 = alpha * alpha
    p2 = alpha * gamma
    p3 = alpha * alpha * gamma
    p4 = beta.astype(np.float32)

    in_maps = []
    for k in range(n_cores):
        sl = slice(k * OSH, (k + 1) * OSH)
        bwt = np.ascontiguousarray(bw[sl].T).reshape(NT, 128, OSH)
        NOT = OSH // 128

        def per_tile(vec):
            return np.ascontiguousarray(vec[sl].reshape(NOT, 128).T)  # [128, NOT]

        coef = np.concatenate(
            [per_tile(p1), per_tile(p2), per_tile(p3), per_tile(p4)], axis=1
        ).astype(np.float32)
        in_maps.append({
            "xtp": xtp,
            "bwt": bwt.astype(np.float16),
            "coef": coef,
        })
    return in_maps


_NC_CACHE = {}


def kernel(x, real_weight, gamma, beta):
    global LAST_RESULTS
    x = np.asarray(x, dtype=np.float32)
    w = np.asarray(real_weight, dtype=np.float32)
    gamma = np.asarray(gamma, dtype=np.float32)
    beta = np.asarray(beta, dtype=np.float32)
    B, I = x.shape
    O = w.shape[0]
    OSH = O // N_CORES
    CH = 512

    simple_tail = bool((gamma > 0).all() and (beta == 0).all())
    key = (B, I, OSH, CH, simple_tail)
    if key not in _NC_CACHE:
        _NC_CACHE[key] = build_nc(B, I, OSH, CH, simple_tail=simple_tail)
    nc = _NC_CACHE[key]

    in_maps = prep_inputs(x, w, gamma, beta, N_CORES, CH)
    trace = bool(int(os.environ.get("KERNEL_TRACE", "0")))
    res = run_bass_kernel_spmd(
        nc, in_maps, core_ids=list(range(N_CORES)), trace=trace
    )
    LAST_RESULTS = res

    NOT = OSH // 128
    plan, _ = slab_plan(B, NOT, CH, simple_tail)
    out = np.empty((B, O), dtype=np.float32)
    for k in range(N_CORES):
        o = res.results[k]["out"].astype(np.float32)   # [OSH, B] int8
        for ot, st, wd, srk, eng in plan:
            if eng != "dve":
                continue
            slb = o[ot * 128 : (ot + 1) * 128, st : st + wd]
            np.multiply(slb, 2.0, out=slb)
            np.subtract(slb, 1.0, out=slb)
        out[:, k * OSH : (k + 1) * OSH] = o.T
    return out
